# revision 28
# baseline (speedup 1.0000x reference)
"""Trainium2 Bass kernel for causal multi-head attention with RoPE.

Problem: B=2, S=2048, D=2048, H=16 heads (HD=128), fp32 reference.

Sharding (8 NeuronCores): 2-way batch x 4-way heads. Core c handles
batch c//4 and heads 4*(c%4) .. 4*(c%4)+4. Each core computes a partial
output projection over its 512-wide head slice; the host sums the 4
partials per batch element (the row-parallel wo all-reduce).

v2 changes vs the 370us baseline (all aimed at DMA efficiency + PE
streaming):
  - All inputs repacked host-side into SBUF-image layouts so every DMA
    moves 4-16KB contiguous per partition line (vs 1KB before) in a
    handful of dma_starts (vs 259): each dma_start costs ~610ns of
    serial SyncE issue time and 1KB packets cap HBM at ~190GB/s.
  - Output written as full 128-token rows [128, 2048] (4KB lines).
  - Causal trim of the diagonal super-block: scores/exp/PV only touch
    the valid q-range of each diagonal k-tile (saves ~10us PE, ~20us
    ACT exp); one shared [128,128] triangular mask replaces the
    [128,4,512] binary mask.
  - Per-chunk pooled q/ctx tiles; PSUM banks: proj+wo share 3, scores
    2, ctx 2, denominator 1.
"""

import math

import numpy as np
import ml_dtypes

import concourse.bass as bass
import concourse.mybir as mybir
import concourse.tile as tile
from concourse import bacc, bass_isa, bass_utils

BF16 = ml_dtypes.bfloat16
F32 = mybir.dt.float32
BF = mybir.dt.bfloat16

B, S, D, H = 2, 2048, 2048, 16
HD = 128
NCORE = 8
HPC = 4            # heads per core
OSL = HPC * HD     # 512-wide output slice per core
NT = S // 128      # 16 token tiles
ND = D // 128      # 16 contraction tiles
NCH = 4            # 512-wide token chunks
SCALE = 1.0 / math.sqrt(HD)


def _build_program():
    nc = bacc.Bacc(
        "TRN2",
        target_bir_lowering=False,
        debug=False,
        enable_asserts=False,
        num_devices=NCORE,
    )
    # All images are laid out so a DMA line (per-partition contiguous
    # run) is >=4KB: [128, K] with K*2B contiguous per partition.
    xr = nc.dram_tensor("xr", [4 * 128, ND * 512], BF, kind="ExternalInput").ap()
    wqr = nc.dram_tensor("wqr", [128, ND * 512], BF, kind="ExternalInput").ap()
    wkr = nc.dram_tensor("wkr", [128, ND * 512], BF, kind="ExternalInput").ap()
    wvr = nc.dram_tensor("wvr", [128, ND * 512], BF, kind="ExternalInput").ap()
    wor = nc.dram_tensor("wor", [128, HPC * D], BF, kind="ExternalInput").ap()
    cos2 = nc.dram_tensor("cos2", [128, S], BF, kind="ExternalInput").ap()
    sin2 = nc.dram_tensor("sin2", [128, S], BF, kind="ExternalInput").ap()
    trim = nc.dram_tensor("trim", [128, 128], BF, kind="ExternalInput").ap()
    out = nc.dram_tensor("out", [S, D], BF, kind="ExternalOutput").ap()

    with tile.TileContext(nc) as tc:
        _kernel_body(tc, xr, wqr, wkr, wvr, wor, cos2, sin2, trim, out)
    nc.compile()
    return nc


def _kernel_body(tc, xr, wqr, wkr, wvr, wor, cos2, sin2, trim, out):
    nc = tc.nc
    Exp = mybir.ActivationFunctionType.Exp

    with (
        tc.tile_pool(name="weights", bufs=1) as wpool,
        tc.tile_pool(name="kv", bufs=1) as kvpool,
        tc.tile_pool(name="consts", bufs=1) as cpool,
        tc.tile_pool(name="qchunk", bufs=2) as qpool,
        tc.tile_pool(name="ctxsb", bufs=2) as ctxpool,
        tc.tile_pool(name="xtp", bufs=2) as xpool,
        tc.tile_pool(name="rope", bufs=3) as rpool,
        tc.tile_pool(name="pg", bufs=6) as ppool,
        tc.tile_pool(name="pgs", bufs=2) as pspool,
        tc.tile_pool(name="den", bufs=2) as dpool,
        tc.tile_pool(name="smallsb", bufs=3) as spool,
        tc.tile_pool(name="outsb", bufs=2) as outpool,
        tc.tile_pool(name="peps", bufs=3, space="PSUM") as peps,
        tc.tile_pool(name="scps", bufs=2, space="PSUM") as scpool,
        tc.tile_pool(name="ctxps", bufs=3, space="PSUM") as ctxps_pool,
    ):
        wq_s = wpool.tile([128, ND * 512], BF, tag="wq")
        wk_s = wpool.tile([128, ND * 512], BF, tag="wk")
        wv_s = wpool.tile([128, ND * 512], BF, tag="wv")
        wo_s = wpool.tile([128, HPC * D], BF, tag="wo")
        cos_s = cpool.tile([128, S], BF, tag="cos")
        sin_s = cpool.tile([128, S], BF, tag="sin")
        tri_s = cpool.tile([128, 128], BF, tag="trim")
        # ones matrix: den matmul broadcasts the k-colsum to all 128
        # partitions, so no cross-partition broadcast hop is needed.
        ones_s = cpool.tile([128, 128], BF, tag="ones")
        nc.gpsimd.memset(ones_s[:], 1.0)
        # Warmup: the PE HAM clock gate needs ~3.4us of sustained busy to
        # lift the 1.2GHz throttle. Run throwaway matmuls on memset data
        # while the first weight/x DMAs are still in flight so the real
        # chains start at 2.4GHz (~13us of DMA lead time to fill).
        warm_in = cpool.tile([128, 512], BF, tag="warm")
        nc.gpsimd.memset(warm_in[:], 0.0)
        warm_ps = ctxps_pool.tile([128, 512], F32, tag="ctxps",
                                  name="warmps")
        for i in range(12):
            nc.tensor.matmul(warm_ps[:], ones_s[:], warm_in[:],
                             start=True, stop=True)
        kt = [kvpool.tile([128, S], BF, tag=f"kt{h}", name=f"kt{h}")
              for h in range(HPC)]
        v_s = kvpool.tile([128, NT * 512], BF, tag="v")

        for qc in range(NCH):
            ch = slice(qc * 512, (qc + 1) * 512)
            xt = xpool.tile([128, ND * 512], BF, tag="xt", name=f"xt{qc}")
            if qc == 0:
                # Startup loads. wq/wk are head-major images so head m's
                # chain only needs its own 512KB piece; xt pieces land
                # right behind wq-m0 so the first chain is fed in order.
                # Two HWDGE FIFOs in parallel: x pieces + rope tables on
                # the scalar queue, weights on the sync queue, so the Q
                # chains' inputs arrive in roughly half the time.
                for p in range(4):
                    nc.scalar.dma_start(xt[:, p * 2048:(p + 1) * 2048],
                                        xr[0:128, p * 2048:(p + 1) * 2048])
                nc.scalar.dma_start(cos_s[:], cos2[:])
                nc.scalar.dma_start(sin_s[:], sin2[:])
                nc.scalar.dma_start(tri_s[:], trim[:])
                for m in range(4):
                    nc.sync.dma_start(wq_s[:, m * 2048:(m + 1) * 2048],
                                      wqr[:, m * 2048:(m + 1) * 2048])
                for m in range(4):
                    nc.sync.dma_start(wk_s[:, m * 2048:(m + 1) * 2048],
                                      wkr[:, m * 2048:(m + 1) * 2048])
            else:
                nc.sync.dma_start(xt[:], xr[qc * 128:(qc + 1) * 128, :])

            # ---- per-chunk emission helpers ----------------------------
            noff = 4 * qc
            qt = []
            ctxT = []
            att = {}  # per-head attention state: (ctx_ps, accq)

            def emit_proj(m, is_q):
                w_s = wq_s if is_q else wk_s
                nm = "q" if is_q else "k"
                ps = peps.tile([128, 512], F32, tag="peps",
                               name=f"ps{nm}{qc}_{m}")
                for d in range(ND):
                    nc.tensor.matmul(
                        ps[:],
                        w_s[:, m * 2048 + d * 128:m * 2048 + (d + 1) * 128],
                        xt[:, d * 512:(d + 1) * 512],
                        start=(d == 0), stop=(d == ND - 1))
                raw = rpool.tile([128, 512], BF, tag="rraw",
                                 name=f"raw{nm}{qc}_{m}")
                nc.scalar.copy(raw[:], ps[:])
                # rope rotate-half: swap 64-partition halves via DMA
                swp = rpool.tile([128, 512], BF, tag="rswp",
                                 name=f"swp{nm}{qc}_{m}")
                nc.sync.dma_start(swp[0:64, :], raw[64:128, :])
                nc.sync.dma_start(swp[64:128, :], raw[0:64, :])
                t1 = rpool.tile([128, 512], BF, tag="rt1")
                nc.vector.tensor_mul(t1[:], raw[:], cos_s[:, ch])
                nc.vector.tensor_mul(swp[:], swp[:], sin_s[:, ch])
                if is_q:
                    dst = qpool.tile([128, 512], BF, tag=f"qt{m}",
                                     name=f"qt{m}_{qc}")
                    qt.append(dst)
                    nc.vector.tensor_add(dst[:], t1[:], swp[:])
                else:
                    nc.vector.tensor_add(kt[m][:, ch], t1[:], swp[:])

            def emit_v(tt):
                pv = peps.tile([128, 512], F32, tag="peps",
                               name=f"psv{qc}_{tt}")
                for d in range(ND):
                    nc.tensor.matmul(
                        pv[:],
                        xt[:, d * 512 + tt * 128:d * 512 + (tt + 1) * 128],
                        wv_s[:, d * 512:(d + 1) * 512],
                        start=(d == 0), stop=(d == ND - 1))
                j = qc * 4 + tt
                nc.vector.tensor_copy(v_s[:, j * 512:(j + 1) * 512], pv[:])

            def emit_off(h):
                # off-diagonal k-tiles (full 512-wide, no mask)
                ctx_ps = ctxps_pool.tile([128, 512], F32, tag="ctxps",
                                         name=f"ctxps{h}_{qc}")
                pg_hist = {}
                pair = [None, None]
                accq = None
                for j in range(noff):
                    sc = scpool.tile([128, 512], F32, tag="sc",
                                     name=f"sc{h}_{qc}_{j}")
                    nc.tensor.matmul(
                        sc[:], kt[h][:, j * 128:(j + 1) * 128],
                        qt[h][:], start=True, stop=True)
                    pg = ppool.tile([128, 512], BF, tag="pg",
                                    name=f"pg{h}_{qc}_{j}")
                    nc.scalar.activation(pg[:], sc[:], Exp, scale=SCALE)
                    nc.tensor.matmul(
                        ctx_ps[:],
                        v_s[:, j * 512 + h * 128:j * 512 + (h + 1) * 128],
                        pg[:], start=(j == 0), stop=False)
                    # denominator pre-sums on DVE: pairs -> quads -> fold
                    pg_hist[j] = pg
                    if j % 2 == 1:
                        pp = pspool.tile([128, 512], BF,
                                         tag=f"pgs{(j // 2) % 2}",
                                         name=f"pgs{h}_{qc}_{j}")
                        nc.vector.tensor_add(pp[:], pg_hist[j - 1][:], pg[:])
                        pair[(j // 2) % 2] = pp
                    if j % 4 == 3:
                        p4 = pspool.tile([128, 512], BF, tag="pgs4",
                                         name=f"pgs4_{h}_{qc}_{j}")
                        nc.vector.tensor_add(p4[:], pair[0][:], pair[1][:])
                        if accq is None:
                            accq = p4
                        else:
                            nacc = pspool.tile([128, 512], BF, tag="pgs8",
                                               name=f"pgs8_{h}_{qc}_{j}")
                            nc.vector.tensor_add(nacc[:], accq[:], p4[:])
                            accq = nacc
                att[h] = (ctx_ps, accq)

            def emit_diag(h):
                # diagonal k-tiles, causally trimmed: k-tile r only sees
                # q columns >= 128r; the first 128 of those are the
                # triangular boundary block (element mask).
                ctx_ps, accq = att[h]
                p4d = dpool.tile([128, 512], F32, tag="p4d",
                                 name=f"p4d{h}_{qc}")
                for r in range(4):
                    off = 128 * r
                    j = noff + r
                    sc = scpool.tile([128, 512], F32, tag="sc",
                                     name=f"scd{h}_{qc}_{r}")
                    nc.tensor.matmul(
                        sc[:, off:512], kt[h][:, j * 128:(j + 1) * 128],
                        qt[h][:, off:512], start=True, stop=True)
                    pg = ppool.tile([128, 512], BF, tag="pg",
                                    name=f"pgd{h}_{qc}_{r}")
                    nc.scalar.activation(pg[:, off:512], sc[:, off:512],
                                         Exp, scale=SCALE)
                    nc.vector.tensor_mul(pg[:, off:off + 128],
                                         pg[:, off:off + 128], tri_s[:])
                    nc.tensor.matmul(
                        ctx_ps[:, off:512],
                        v_s[:, j * 512 + h * 128:j * 512 + (h + 1) * 128],
                        pg[:, off:512],
                        start=(qc == 0 and r == 0), stop=(r == 3))
                    if r == 0:
                        nc.vector.tensor_copy(p4d[:], pg[:])
                    else:
                        nc.vector.tensor_add(p4d[:, off:512],
                                             p4d[:, off:512], pg[:, off:512])
                # denominator: k-colsum broadcast to all partitions via a
                # GpSimd all-reduce (keeps the PE free of den matmuls)
                if accq is not None:
                    dtot = dpool.tile([128, 512], F32, tag="dtot",
                                      name=f"dtot{h}_{qc}")
                    nc.vector.tensor_add(dtot[:], accq[:], p4d[:])
                else:
                    dtot = p4d
                den_bc = spool.tile([128, 512], F32, tag="denbc")
                nc.gpsimd.partition_all_reduce(
                    den_bc[:], dtot[:], 128, bass_isa.ReduceOp.add)
                rbc = spool.tile([128, 512], F32, tag="rbc")
                nc.vector.reciprocal_approx_fast(rbc[:], den_bc[:])
                ctx = ctxpool.tile([128, 512], BF, tag=f"ctx{h}",
                                   name=f"ctxT{h}_{qc}")
                ctxT.append(ctx)
                nc.vector.tensor_mul(ctx[:], ctx_ps[:], rbc[:])

            # ---- emission order ----------------------------------------
            if qc < NCH - 1:
                for m in range(HPC):
                    emit_proj(m, True)
                    emit_proj(m, False)
                if qc == 0:
                    # wv/wo are needed later; issue after the Q/K chains so
                    # chunk-0 rope swaps aren't stuck behind them on the queue
                    nc.sync.dma_start(wv_s[:, 0:4096], wvr[:, 0:4096])
                    nc.sync.dma_start(wv_s[:, 4096:8192], wvr[:, 4096:8192])
                    nc.sync.dma_start(wo_s[:], wor[:])
                for tt in range(4):
                    emit_v(tt)
                # prefetch next chunk's x while attention runs
                xt_next = xpool.tile([128, ND * 512], BF, tag="xt",
                                     name=f"xt{qc + 1}")
                nc.sync.dma_start(
                    xt_next[:], xr[(qc + 1) * 128:(qc + 2) * 128, :])
                for h in range(HPC):
                    emit_off(h)
                    emit_diag(h)
            else:
                # last chunk: no next-chunk projections exist to hide the
                # ACT-bound softmax, so interleave this chunk's own K/V
                # chains with the off-diagonal attention of heads 0/1
                for m in range(HPC):
                    emit_proj(m, True)
                emit_proj(0, False)
                emit_proj(1, False)
                emit_off(0)
                emit_proj(2, False)
                emit_proj(3, False)
                emit_off(1)
                for tt in range(4):
                    emit_v(tt)
                emit_diag(0)
                emit_diag(1)
                emit_off(2)
                emit_off(3)
                emit_diag(2)
                emit_diag(3)

            # ---- output projection for chunk qc ------------------------
            # Groups are staggered: each group's first 3 head-MMs are
            # emitted two groups ahead of its final head-3 MM, so the PE
            # has ready work while the last head's softmax tail (den ->
            # recip -> normalize) resolves.
            STAG = 2 if qc < NCH - 1 else 3
            groups = [(tl, dc) for tl in range(4) for dc in range(4)]
            opst = {}
            osbt = {}
            for gi in range(16 + STAG):
                if gi >= STAG:
                    tl, dc = groups[gi - STAG]
                    tt = 4 * qc + tl
                    ops = opst[gi - STAG]
                    osb = osbt[tl]
                    nc.tensor.matmul(
                        ops[:], ctxT[3][:, tl * 128:(tl + 1) * 128],
                        wo_s[:, 3 * D + dc * 512:3 * D + (dc + 1) * 512],
                        start=False, stop=True)
                    if dc % 2 == 0:
                        nc.vector.tensor_copy(
                            osb[:, dc * 512:(dc + 1) * 512], ops[:])
                    else:
                        nc.scalar.copy(
                            osb[:, dc * 512:(dc + 1) * 512], ops[:])
                    # write each half-row as soon as its evacs are done
                    if dc == 1:
                        nc.sync.dma_start(
                            out[tt * 128:(tt + 1) * 128, 0:1024],
                            osb[:, 0:1024])
                    elif dc == 3:
                        nc.sync.dma_start(
                            out[tt * 128:(tt + 1) * 128, 1024:2048],
                            osb[:, 1024:2048])
                if gi < 16:
                    tl, dc = groups[gi]
                    if dc == 0:
                        osbt[tl] = outpool.tile([128, D], BF, tag="osb",
                                                name=f"osb{4 * qc + tl}")
                    # ops tiles live in the scores pool (disjoint in time
                    # with attention) so proj(qc+1) never waits on wo
                    # slots; on the last chunk use the idle proj pool
                    # instead, leaving the sc pool to head-3's scores so
                    # wo partials can overlap the tail of attention.
                    if qc == NCH - 1:
                        ops = peps.tile([128, 512], F32, tag="peps",
                                        name=f"ops{qc}_{tl}_{dc}")
                    else:
                        ops = scpool.tile([128, 512], F32, tag="sc",
                                          name=f"ops{qc}_{tl}_{dc}")
                    opst[gi] = ops
                    for e in range(3):
                        nc.tensor.matmul(
                            ops[:], ctxT[e][:, tl * 128:(tl + 1) * 128],
                            wo_s[:, e * D + dc * 512:e * D + (dc + 1) * 512],
                            start=(e == 0), stop=False)


def _host_prep(x, freqs_cos, freqs_sin, mask, wq, wk, wv, wo):
    """Build per-core input dicts (SBUF-image layouts, bf16)."""
    x = np.asarray(x, np.float32)
    wq = np.asarray(wq, np.float32)
    wk = np.asarray(wk, np.float32)
    wv = np.asarray(wv, np.float32)
    wo = np.asarray(wo, np.float32)
    cos = np.asarray(freqs_cos, np.float32)
    sin = np.asarray(freqs_sin, np.float32)

    perm = np.concatenate([np.arange(0, HD, 2), np.arange(1, HD, 2)])
    cos2 = np.ascontiguousarray(
        np.concatenate([cos.T, cos.T], axis=0)).astype(BF16)
    sin2 = np.ascontiguousarray(
        np.concatenate([-sin.T, sin.T], axis=0)).astype(BF16)

    # triangular boundary mask: T[kl, ql] = 1 iff ql >= kl
    kl = np.arange(128)[:, None]
    ql = np.arange(128)[None, :]
    trim = (ql >= kl).astype(np.float32).astype(BF16)

    def img_dxk(wT):
        # [D, K] -> [128, ND*K] with [p, d*K+c] = wT[d*128+p, c]
        Dd, K = wT.shape
        return np.ascontiguousarray(
            wT.reshape(ND, 128, K).transpose(1, 0, 2).reshape(128, ND * K))

    def img_head_major(wT):
        # [D, 512] -> [128, 8192] with [p, m*2048 + d*128 + c] =
        # wT[d*128+p, m*128+c]; head m's chain reads a contiguous 512KB
        return np.ascontiguousarray(
            wT.reshape(ND, 128, HPC, 128).transpose(1, 2, 0, 3).reshape(
                128, ND * 512))

    in_maps = []
    for c in range(NCORE):
        b = c // 4
        o0 = OSL * (c % 4)
        rows = np.concatenate(
            [o0 + h * HD + perm for h in range(HPC)])
        xT = np.ascontiguousarray(x[b].T)  # [D, S]
        # x image: [4*128, ND*512]: [qc*128+p, d*512+t] = xT[d*128+p, qc*512+t]
        xi = xT.reshape(ND, 128, 4, 512).transpose(2, 1, 0, 3).reshape(
            4 * 128, ND * 512)
        # wo image: [128, HPC*D]: [p, e*D+c] = woT[e*128+p, c]
        woT = wo[:, o0:o0 + OSL].T  # [512, D]
        woi = woT.reshape(HPC, 128, D).transpose(1, 0, 2).reshape(128, HPC * D)
        in_maps.append(dict(
            xr=np.ascontiguousarray(xi).astype(BF16),
            wqr=img_head_major(wq[rows].T).astype(BF16),
            wkr=img_head_major(wk[rows].T).astype(BF16),
            wvr=img_dxk(wv[o0:o0 + OSL].T).astype(BF16),
            wor=np.ascontiguousarray(woi).astype(BF16),
            cos2=cos2, sin2=sin2, trim=trim,
        ))
    return in_maps


_NC_CACHE = None


def get_program():
    global _NC_CACHE
    if _NC_CACHE is None:
        _NC_CACHE = _build_program()
    return _NC_CACHE


def run_on_cores(in_maps, trace=False):
    nc = get_program()
    return bass_utils.run_bass_kernel_spmd(
        nc, in_maps, core_ids=list(range(NCORE)), trace=trace)


def kernel(x, freqs_cos, freqs_sin, mask, wq, wk, wv, wo, start_pos=0,
           **_ignored):
    in_maps = _host_prep(x, freqs_cos, freqs_sin, mask, wq, wk, wv, wo)
    res = run_on_cores(in_maps, trace=False)
    outs = [res.results[c]["out"] for c in range(NCORE)]
    full = np.empty((B, S, D), np.float32)
    for b in range(B):
        acc = outs[4 * b].astype(np.float32)
        for c in range(4 * b + 1, 4 * b + 4):
            acc = acc + outs[c]
        full[b] = acc
    return full


# revision 33
# speedup vs baseline: 1.3394x; 1.3394x over previous
"""Trainium2 Bass kernel for causal multi-head attention with RoPE.

Problem: B=2, S=2048, D=2048, H=16 heads (HD=128), fp32 reference.

Sharding (8 NeuronCores): 2-way batch x 4-way heads. Core c handles
batch c//4 and heads 4*(c%4) .. 4*(c%4)+4. Each core computes a partial
output projection over its 512-wide head slice; the host sums the 4
partials per batch element (the row-parallel wo all-reduce).

v2 changes vs the 370us baseline (all aimed at DMA efficiency + PE
streaming):
  - All inputs repacked host-side into SBUF-image layouts so every DMA
    moves 4-16KB contiguous per partition line (vs 1KB before) in a
    handful of dma_starts (vs 259): each dma_start costs ~610ns of
    serial SyncE issue time and 1KB packets cap HBM at ~190GB/s.
  - Output written as full 128-token rows [128, 2048] (4KB lines).
  - Causal trim of the diagonal super-block: scores/exp/PV only touch
    the valid q-range of each diagonal k-tile (saves ~10us PE, ~20us
    ACT exp); one shared [128,128] triangular mask replaces the
    [128,4,512] binary mask.
  - Per-chunk pooled q/ctx tiles; PSUM banks: proj+wo share 3, scores
    2, ctx 2, denominator 1.
"""

import math

import numpy as np
import ml_dtypes

import concourse.bass as bass
import concourse.mybir as mybir
import concourse.tile as tile
from concourse import bacc, bass_isa, bass_utils

BF16 = ml_dtypes.bfloat16
F32 = mybir.dt.float32
BF = mybir.dt.bfloat16

B, S, D, H = 2, 2048, 2048, 16
HD = 128
NCORE = 8
HPC = 4            # heads per core
OSL = HPC * HD     # 512-wide output slice per core
NT = S // 128      # 16 token tiles
ND = D // 128      # 16 contraction tiles
NCH = 4            # 512-wide token chunks
SCALE = 1.0 / math.sqrt(HD)


def _build_program():
    nc = bacc.Bacc(
        "TRN2",
        target_bir_lowering=False,
        debug=False,
        enable_asserts=False,
        num_devices=NCORE,
    )
    # All images are laid out so a DMA line (per-partition contiguous
    # run) is >=4KB: [128, K] with K*2B contiguous per partition.
    xr = nc.dram_tensor("xr", [4 * 128, ND * 512], BF, kind="ExternalInput").ap()
    wqr = nc.dram_tensor("wqr", [128, ND * 512], BF, kind="ExternalInput").ap()
    wkr = nc.dram_tensor("wkr", [128, ND * 512], BF, kind="ExternalInput").ap()
    wvr = nc.dram_tensor("wvr", [128, ND * 512], BF, kind="ExternalInput").ap()
    wor = nc.dram_tensor("wor", [128, HPC * D], BF, kind="ExternalInput").ap()
    cos2 = nc.dram_tensor("cos2", [128, S], BF, kind="ExternalInput").ap()
    sin2 = nc.dram_tensor("sin2", [128, S], BF, kind="ExternalInput").ap()
    trim = nc.dram_tensor("trim", [128, 128], BF, kind="ExternalInput").ap()
    out = nc.dram_tensor("out", [S, D], BF, kind="ExternalOutput").ap()

    with tile.TileContext(nc) as tc:
        _kernel_body(tc, xr, wqr, wkr, wvr, wor, cos2, sin2, trim, out)
    nc.compile()
    return nc


def _kernel_body(tc, xr, wqr, wkr, wvr, wor, cos2, sin2, trim, out):
    nc = tc.nc
    Exp = mybir.ActivationFunctionType.Exp

    with (
        tc.tile_pool(name="weights", bufs=1) as wpool,
        tc.tile_pool(name="kv", bufs=1) as kvpool,
        tc.tile_pool(name="consts", bufs=1) as cpool,
        tc.tile_pool(name="qchunk", bufs=2) as qpool,
        tc.tile_pool(name="ctxsb", bufs=2) as ctxpool,
        tc.tile_pool(name="xtp", bufs=2) as xpool,
        tc.tile_pool(name="rope", bufs=3) as rpool,
        tc.tile_pool(name="pg", bufs=6) as ppool,
        tc.tile_pool(name="pgs", bufs=2) as pspool,
        tc.tile_pool(name="den", bufs=2) as dpool,
        tc.tile_pool(name="smallsb", bufs=3) as spool,
        tc.tile_pool(name="outsb", bufs=2) as outpool,
        tc.tile_pool(name="peps", bufs=3, space="PSUM") as peps,
        tc.tile_pool(name="scps", bufs=2, space="PSUM") as scpool,
        tc.tile_pool(name="denps", bufs=1, space="PSUM") as denpool,
        tc.tile_pool(name="ctxps", bufs=2, space="PSUM") as ctxps_pool,
    ):
        wq_s = wpool.tile([128, ND * 512], BF, tag="wq")
        wk_s = wpool.tile([128, ND * 512], BF, tag="wk")
        wv_s = wpool.tile([128, ND * 512], BF, tag="wv")
        wo_s = wpool.tile([128, HPC * D], BF, tag="wo")
        cos_s = cpool.tile([128, S], BF, tag="cos")
        sin_s = cpool.tile([128, S], BF, tag="sin")
        tri_s = cpool.tile([128, 128], BF, tag="trim")
        # ones matrix: den matmul broadcasts the k-colsum to all 128
        # partitions, so no cross-partition broadcast hop is needed.
        ones_s = cpool.tile([128, 128], BF, tag="ones")
        nc.gpsimd.memset(ones_s[:], 1.0)
        # Warmup: the PE HAM clock gate needs ~3.4us of sustained busy to
        # lift the 1.2GHz throttle. Run throwaway matmuls on memset data
        # while the first weight/x DMAs are still in flight so the real
        # chains start at 2.4GHz (~13us of DMA lead time to fill).
        warm_in = cpool.tile([128, 512], BF, tag="warm")
        nc.gpsimd.memset(warm_in[:], 0.0)
        warm_ps = denpool.tile([128, 512], F32, tag="den", name="warmps")
        for i in range(12):
            nc.tensor.matmul(warm_ps[:], ones_s[:], warm_in[:],
                             start=True, stop=True)
        kt = [kvpool.tile([128, S], BF, tag=f"kt{h}", name=f"kt{h}")
              for h in range(HPC)]
        v_s = kvpool.tile([128, NT * 512], BF, tag="v")

        for qc in range(NCH):
            ch = slice(qc * 512, (qc + 1) * 512)
            xt = xpool.tile([128, ND * 512], BF, tag="xt", name=f"xt{qc}")
            if qc == 0:
                # Startup loads. wq/wk are head-major images so head m's
                # chain only needs its own 512KB piece; xt pieces land
                # right behind wq-m0 so the first chain is fed in order.
                # Two HWDGE FIFOs in parallel: x pieces + rope tables on
                # the scalar queue, weights on the sync queue, so the Q
                # chains' inputs arrive in roughly half the time.
                for p in range(4):
                    nc.scalar.dma_start(xt[:, p * 2048:(p + 1) * 2048],
                                        xr[0:128, p * 2048:(p + 1) * 2048])
                nc.scalar.dma_start(cos_s[:], cos2[:])
                nc.scalar.dma_start(sin_s[:], sin2[:])
                nc.scalar.dma_start(tri_s[:], trim[:])
                for m in range(4):
                    nc.sync.dma_start(wq_s[:, m * 2048:(m + 1) * 2048],
                                      wqr[:, m * 2048:(m + 1) * 2048])
                for m in range(4):
                    nc.sync.dma_start(wk_s[:, m * 2048:(m + 1) * 2048],
                                      wkr[:, m * 2048:(m + 1) * 2048])
            else:
                nc.sync.dma_start(xt[:], xr[qc * 128:(qc + 1) * 128, :])

            # ---- per-chunk emission helpers ----------------------------
            noff = 4 * qc
            qt = []
            ctxT = []
            att = {}  # per-head attention state: (ctx_ps, accq)

            def emit_proj(m, is_q):
                w_s = wq_s if is_q else wk_s
                nm = "q" if is_q else "k"
                ps = peps.tile([128, 512], F32, tag="peps",
                               name=f"ps{nm}{qc}_{m}")
                for d in range(ND):
                    nc.tensor.matmul(
                        ps[:],
                        w_s[:, m * 2048 + d * 128:m * 2048 + (d + 1) * 128],
                        xt[:, d * 512:(d + 1) * 512],
                        start=(d == 0), stop=(d == ND - 1))
                raw = rpool.tile([128, 512], BF, tag="rraw",
                                 name=f"raw{nm}{qc}_{m}")
                nc.scalar.copy(raw[:], ps[:])
                # rope rotate-half: swap 64-partition halves via DMA
                swp = rpool.tile([128, 512], BF, tag="rswp",
                                 name=f"swp{nm}{qc}_{m}")
                nc.sync.dma_start(swp[0:64, :], raw[64:128, :])
                nc.sync.dma_start(swp[64:128, :], raw[0:64, :])
                t1 = rpool.tile([128, 512], BF, tag="rt1")
                nc.vector.tensor_mul(t1[:], raw[:], cos_s[:, ch])
                nc.vector.tensor_mul(swp[:], swp[:], sin_s[:, ch])
                if is_q:
                    dst = qpool.tile([128, 512], BF, tag=f"qt{m}",
                                     name=f"qt{m}_{qc}")
                    qt.append(dst)
                    nc.vector.tensor_add(dst[:], t1[:], swp[:])
                else:
                    nc.vector.tensor_add(kt[m][:, ch], t1[:], swp[:])

            def emit_v(tt):
                pv = peps.tile([128, 512], F32, tag="peps",
                               name=f"psv{qc}_{tt}")
                for d in range(ND):
                    nc.tensor.matmul(
                        pv[:],
                        xt[:, d * 512 + tt * 128:d * 512 + (tt + 1) * 128],
                        wv_s[:, d * 512:(d + 1) * 512],
                        start=(d == 0), stop=(d == ND - 1))
                j = qc * 4 + tt
                nc.vector.tensor_copy(v_s[:, j * 512:(j + 1) * 512], pv[:])

            def emit_off(h):
                # off-diagonal k-tiles (full 512-wide, no mask)
                ctx_ps = ctxps_pool.tile([128, 512], F32, tag="ctxps",
                                         name=f"ctxps{h}_{qc}")
                den_ps = denpool.tile([128, 512], F32, tag="den",
                                      name=f"den{h}_{qc}")
                att[h] = (ctx_ps, den_ps)
                pg_hist = {}
                pair = [None, None]
                accq = None
                for j in range(noff):
                    sc = scpool.tile([128, 512], F32, tag="sc",
                                     name=f"sc{h}_{qc}_{j}")
                    nc.tensor.matmul(
                        sc[:], kt[h][:, j * 128:(j + 1) * 128],
                        qt[h][:], start=True, stop=True)
                    pg = ppool.tile([128, 512], BF, tag="pg",
                                    name=f"pg{h}_{qc}_{j}")
                    nc.scalar.activation(pg[:], sc[:], Exp, scale=SCALE)
                    nc.tensor.matmul(
                        ctx_ps[:],
                        v_s[:, j * 512 + h * 128:j * 512 + (h + 1) * 128],
                        pg[:], start=(j == 0), stop=False)
                    # denominator pre-sums on DVE: pairs -> quads -> fold
                    pg_hist[j] = pg
                    if j % 2 == 1:
                        pp = pspool.tile([128, 512], BF,
                                         tag=f"pgs{(j // 2) % 2}",
                                         name=f"pgs{h}_{qc}_{j}")
                        nc.vector.tensor_add(pp[:], pg_hist[j - 1][:], pg[:])
                        pair[(j // 2) % 2] = pp
                    if j % 4 == 3:
                        p4 = pspool.tile([128, 512], BF, tag="pgs4",
                                         name=f"pgs4_{h}_{qc}_{j}")
                        nc.vector.tensor_add(p4[:], pair[0][:], pair[1][:])
                        if accq is None:
                            accq = p4
                        else:
                            nacc = pspool.tile([128, 512], BF, tag="pgs8",
                                               name=f"pgs8_{h}_{qc}_{j}")
                            nc.vector.tensor_add(nacc[:], accq[:], p4[:])
                            accq = nacc
                if accq is not None:
                    nc.tensor.matmul(den_ps[:], ones_s[:], accq[:],
                                     start=True, stop=False)

            def emit_diag(h):
                # diagonal k-tiles, causally trimmed: k-tile r only sees
                # q columns >= 128r; the first 128 of those are the
                # triangular boundary block (element mask).
                ctx_ps, den_ps = att[h]
                p4d = dpool.tile([128, 512], BF, tag="p4d",
                                 name=f"p4d{h}_{qc}")
                for r in range(4):
                    off = 128 * r
                    j = noff + r
                    sc = scpool.tile([128, 512], F32, tag="sc",
                                     name=f"scd{h}_{qc}_{r}")
                    nc.tensor.matmul(
                        sc[:, off:512], kt[h][:, j * 128:(j + 1) * 128],
                        qt[h][:, off:512], start=True, stop=True)
                    pg = ppool.tile([128, 512], BF, tag="pg",
                                    name=f"pgd{h}_{qc}_{r}")
                    nc.scalar.activation(pg[:, off:512], sc[:, off:512],
                                         Exp, scale=SCALE)
                    nc.vector.tensor_mul(pg[:, off:off + 128],
                                         pg[:, off:off + 128], tri_s[:])
                    nc.tensor.matmul(
                        ctx_ps[:, off:512],
                        v_s[:, j * 512 + h * 128:j * 512 + (h + 1) * 128],
                        pg[:, off:512],
                        start=(qc == 0 and r == 0), stop=(r == 3))
                    if r == 0:
                        nc.vector.tensor_copy(p4d[:], pg[:])
                    else:
                        nc.vector.tensor_add(p4d[:, off:512],
                                             p4d[:, off:512], pg[:, off:512])
                nc.tensor.matmul(den_ps[:], ones_s[:], p4d[:],
                                 start=(qc == 0), stop=True)
                # softmax normalization folded into ctx eviction; den is
                # already broadcast across partitions by the ones matmul
                rbc = spool.tile([128, 512], F32, tag="rbc")
                nc.vector.reciprocal_approx_fast(rbc[:], den_ps[:])
                ctx = ctxpool.tile([128, 512], BF, tag=f"ctx{h}",
                                   name=f"ctxT{h}_{qc}")
                ctxT.append(ctx)
                nc.vector.tensor_mul(ctx[:], ctx_ps[:], rbc[:])

            # ---- emission order ----------------------------------------
            if qc < NCH - 1:
                for m in range(HPC):
                    emit_proj(m, True)
                    emit_proj(m, False)
                if qc == 0:
                    # wv/wo are needed later; issue after the Q/K chains so
                    # chunk-0 rope swaps aren't stuck behind them on the queue
                    nc.sync.dma_start(wv_s[:, 0:4096], wvr[:, 0:4096])
                    nc.sync.dma_start(wv_s[:, 4096:8192], wvr[:, 4096:8192])
                    nc.sync.dma_start(wo_s[:], wor[:])
                for tt in range(4):
                    emit_v(tt)
                # prefetch next chunk's x while attention runs
                xt_next = xpool.tile([128, ND * 512], BF, tag="xt",
                                     name=f"xt{qc + 1}")
                nc.sync.dma_start(
                    xt_next[:], xr[(qc + 1) * 128:(qc + 2) * 128, :])
                for h in range(HPC):
                    emit_off(h)
                    emit_diag(h)
            else:
                # last chunk: no next-chunk projections exist to hide the
                # ACT-bound softmax, so interleave this chunk's own K/V
                # chains with the off-diagonal attention of heads 0/1
                for m in range(HPC):
                    emit_proj(m, True)
                emit_proj(0, False)
                emit_proj(1, False)
                emit_off(0)
                emit_proj(2, False)
                emit_proj(3, False)
                emit_off(1)
                for tt in range(4):
                    emit_v(tt)
                emit_diag(0)
                emit_diag(1)
                emit_off(2)
                emit_off(3)
                emit_diag(2)
                emit_diag(3)

            # ---- output projection for chunk qc ------------------------
            # Groups are staggered: each group's first 3 head-MMs are
            # emitted two groups ahead of its final head-3 MM, so the PE
            # has ready work while the last head's softmax tail (den ->
            # recip -> normalize) resolves.
            STAG = 2 if qc < NCH - 1 else 3
            groups = [(tl, dc) for tl in range(4) for dc in range(4)]
            opst = {}
            osbt = {}
            for gi in range(16 + STAG):
                if gi >= STAG:
                    tl, dc = groups[gi - STAG]
                    tt = 4 * qc + tl
                    ops = opst[gi - STAG]
                    osb = osbt[tl]
                    nc.tensor.matmul(
                        ops[:], ctxT[3][:, tl * 128:(tl + 1) * 128],
                        wo_s[:, 3 * D + dc * 512:3 * D + (dc + 1) * 512],
                        start=False, stop=True)
                    if dc % 2 == 0:
                        nc.vector.tensor_copy(
                            osb[:, dc * 512:(dc + 1) * 512], ops[:])
                    else:
                        nc.scalar.copy(
                            osb[:, dc * 512:(dc + 1) * 512], ops[:])
                    # write each half-row as soon as its evacs are done
                    if dc == 1:
                        nc.sync.dma_start(
                            out[tt * 128:(tt + 1) * 128, 0:1024],
                            osb[:, 0:1024])
                    elif dc == 3:
                        nc.sync.dma_start(
                            out[tt * 128:(tt + 1) * 128, 1024:2048],
                            osb[:, 1024:2048])
                if gi < 16:
                    tl, dc = groups[gi]
                    if dc == 0:
                        osbt[tl] = outpool.tile([128, D], BF, tag="osb",
                                                name=f"osb{4 * qc + tl}")
                    # ops tiles live in the scores pool (disjoint in time
                    # with attention) so proj(qc+1) never waits on wo
                    # slots; on the last chunk use the idle proj pool
                    # instead, leaving the sc pool to head-3's scores so
                    # wo partials can overlap the tail of attention.
                    if qc == NCH - 1:
                        ops = peps.tile([128, 512], F32, tag="peps",
                                        name=f"ops{qc}_{tl}_{dc}")
                    else:
                        ops = scpool.tile([128, 512], F32, tag="sc",
                                          name=f"ops{qc}_{tl}_{dc}")
                    opst[gi] = ops
                    for e in range(3):
                        nc.tensor.matmul(
                            ops[:], ctxT[e][:, tl * 128:(tl + 1) * 128],
                            wo_s[:, e * D + dc * 512:e * D + (dc + 1) * 512],
                            start=(e == 0), stop=False)


def _host_prep(x, freqs_cos, freqs_sin, mask, wq, wk, wv, wo):
    """Build per-core input dicts (SBUF-image layouts, bf16)."""
    x = np.asarray(x, np.float32)
    wq = np.asarray(wq, np.float32)
    wk = np.asarray(wk, np.float32)
    wv = np.asarray(wv, np.float32)
    wo = np.asarray(wo, np.float32)
    cos = np.asarray(freqs_cos, np.float32)
    sin = np.asarray(freqs_sin, np.float32)

    perm = np.concatenate([np.arange(0, HD, 2), np.arange(1, HD, 2)])
    cos2 = np.ascontiguousarray(
        np.concatenate([cos.T, cos.T], axis=0)).astype(BF16)
    sin2 = np.ascontiguousarray(
        np.concatenate([-sin.T, sin.T], axis=0)).astype(BF16)

    # triangular boundary mask: T[kl, ql] = 1 iff ql >= kl
    kl = np.arange(128)[:, None]
    ql = np.arange(128)[None, :]
    trim = (ql >= kl).astype(np.float32).astype(BF16)

    def img_dxk(wT):
        # [D, K] -> [128, ND*K] with [p, d*K+c] = wT[d*128+p, c]
        Dd, K = wT.shape
        return np.ascontiguousarray(
            wT.reshape(ND, 128, K).transpose(1, 0, 2).reshape(128, ND * K))

    def img_head_major(wT):
        # [D, 512] -> [128, 8192] with [p, m*2048 + d*128 + c] =
        # wT[d*128+p, m*128+c]; head m's chain reads a contiguous 512KB
        return np.ascontiguousarray(
            wT.reshape(ND, 128, HPC, 128).transpose(1, 2, 0, 3).reshape(
                128, ND * 512))

    in_maps = []
    for c in range(NCORE):
        b = c // 4
        o0 = OSL * (c % 4)
        rows = np.concatenate(
            [o0 + h * HD + perm for h in range(HPC)])
        xT = np.ascontiguousarray(x[b].T)  # [D, S]
        # x image: [4*128, ND*512]: [qc*128+p, d*512+t] = xT[d*128+p, qc*512+t]
        xi = xT.reshape(ND, 128, 4, 512).transpose(2, 1, 0, 3).reshape(
            4 * 128, ND * 512)
        # wo image: [128, HPC*D]: [p, e*D+c] = woT[e*128+p, c]
        woT = wo[:, o0:o0 + OSL].T  # [512, D]
        woi = woT.reshape(HPC, 128, D).transpose(1, 0, 2).reshape(128, HPC * D)
        in_maps.append(dict(
            xr=np.ascontiguousarray(xi).astype(BF16),
            wqr=img_head_major(wq[rows].T).astype(BF16),
            wkr=img_head_major(wk[rows].T).astype(BF16),
            wvr=img_dxk(wv[o0:o0 + OSL].T).astype(BF16),
            wor=np.ascontiguousarray(woi).astype(BF16),
            cos2=cos2, sin2=sin2, trim=trim,
        ))
    return in_maps


_NC_CACHE = None


def get_program():
    global _NC_CACHE
    if _NC_CACHE is None:
        _NC_CACHE = _build_program()
    return _NC_CACHE


def run_on_cores(in_maps, trace=False):
    nc = get_program()
    return bass_utils.run_bass_kernel_spmd(
        nc, in_maps, core_ids=list(range(NCORE)), trace=trace)


def kernel(x, freqs_cos, freqs_sin, mask, wq, wk, wv, wo, start_pos=0,
           **_ignored):
    in_maps = _host_prep(x, freqs_cos, freqs_sin, mask, wq, wk, wv, wo)
    res = run_on_cores(in_maps, trace=False)
    outs = [res.results[c]["out"] for c in range(NCORE)]
    full = np.empty((B, S, D), np.float32)
    for b in range(B):
        acc = outs[4 * b].astype(np.float32)
        for c in range(4 * b + 1, 4 * b + 4):
            acc = acc + outs[c]
        full[b] = acc
    return full


# revision 34
# speedup vs baseline: 1.3700x; 1.0228x over previous
"""Trainium2 Bass kernel for causal multi-head attention with RoPE.

Problem: B=2, S=2048, D=2048, H=16 heads (HD=128), fp32 reference.

Sharding (8 NeuronCores): 2-way batch x 4-way heads. Core c handles
batch c//4 and heads 4*(c%4) .. 4*(c%4)+4. Each core computes a partial
output projection over its 512-wide head slice; the host sums the 4
partials per batch element (the row-parallel wo all-reduce).

v2 changes vs the 370us baseline (all aimed at DMA efficiency + PE
streaming):
  - All inputs repacked host-side into SBUF-image layouts so every DMA
    moves 4-16KB contiguous per partition line (vs 1KB before) in a
    handful of dma_starts (vs 259): each dma_start costs ~610ns of
    serial SyncE issue time and 1KB packets cap HBM at ~190GB/s.
  - Output written as full 128-token rows [128, 2048] (4KB lines).
  - Causal trim of the diagonal super-block: scores/exp/PV only touch
    the valid q-range of each diagonal k-tile (saves ~10us PE, ~20us
    ACT exp); one shared [128,128] triangular mask replaces the
    [128,4,512] binary mask.
  - Per-chunk pooled q/ctx tiles; PSUM banks: proj+wo share 3, scores
    2, ctx 2, denominator 1.
"""

import math

import numpy as np
import ml_dtypes

import concourse.bass as bass
import concourse.mybir as mybir
import concourse.tile as tile
from concourse import bacc, bass_isa, bass_utils

BF16 = ml_dtypes.bfloat16
F32 = mybir.dt.float32
BF = mybir.dt.bfloat16

B, S, D, H = 2, 2048, 2048, 16
HD = 128
NCORE = 8
HPC = 4            # heads per core
OSL = HPC * HD     # 512-wide output slice per core
NT = S // 128      # 16 token tiles
ND = D // 128      # 16 contraction tiles
NCH = 4            # 512-wide token chunks
SCALE = 1.0 / math.sqrt(HD)


def _build_program():
    nc = bacc.Bacc(
        "TRN2",
        target_bir_lowering=False,
        debug=False,
        enable_asserts=False,
        num_devices=NCORE,
    )
    # All images are laid out so a DMA line (per-partition contiguous
    # run) is >=4KB: [128, K] with K*2B contiguous per partition.
    xr = nc.dram_tensor("xr", [4 * 128, ND * 512], BF, kind="ExternalInput").ap()
    wqr = nc.dram_tensor("wqr", [128, ND * 512], BF, kind="ExternalInput").ap()
    wkr = nc.dram_tensor("wkr", [128, ND * 512], BF, kind="ExternalInput").ap()
    wvr = nc.dram_tensor("wvr", [128, ND * 512], BF, kind="ExternalInput").ap()
    wor = nc.dram_tensor("wor", [128, HPC * D], BF, kind="ExternalInput").ap()
    cos2 = nc.dram_tensor("cos2", [128, S], BF, kind="ExternalInput").ap()
    sin2 = nc.dram_tensor("sin2", [128, S], BF, kind="ExternalInput").ap()
    trim = nc.dram_tensor("trim", [128, 128], BF, kind="ExternalInput").ap()
    out = nc.dram_tensor("out", [S, D], BF, kind="ExternalOutput").ap()

    with tile.TileContext(nc) as tc:
        _kernel_body(tc, xr, wqr, wkr, wvr, wor, cos2, sin2, trim, out)
    nc.compile()
    return nc


def _kernel_body(tc, xr, wqr, wkr, wvr, wor, cos2, sin2, trim, out):
    nc = tc.nc
    Exp = mybir.ActivationFunctionType.Exp

    with (
        tc.tile_pool(name="weights", bufs=1) as wpool,
        tc.tile_pool(name="kv", bufs=1) as kvpool,
        tc.tile_pool(name="consts", bufs=1) as cpool,
        tc.tile_pool(name="qchunk", bufs=2) as qpool,
        tc.tile_pool(name="ctxsb", bufs=2) as ctxpool,
        tc.tile_pool(name="xtp", bufs=2) as xpool,
        tc.tile_pool(name="rope", bufs=3) as rpool,
        tc.tile_pool(name="pg", bufs=6) as ppool,
        tc.tile_pool(name="pgs", bufs=2) as pspool,
        tc.tile_pool(name="den", bufs=2) as dpool,
        tc.tile_pool(name="smallsb", bufs=3) as spool,
        tc.tile_pool(name="outsb", bufs=2) as outpool,
        tc.tile_pool(name="peps", bufs=3, space="PSUM") as peps,
        tc.tile_pool(name="scps", bufs=2, space="PSUM") as scpool,
        tc.tile_pool(name="denps", bufs=1, space="PSUM") as denpool,
        tc.tile_pool(name="ctxps", bufs=2, space="PSUM") as ctxps_pool,
    ):
        wq_s = wpool.tile([128, ND * 512], BF, tag="wq")
        wk_s = wpool.tile([128, ND * 512], BF, tag="wk")
        wv_s = wpool.tile([128, ND * 512], BF, tag="wv")
        wo_s = wpool.tile([128, HPC * D], BF, tag="wo")
        cos_s = cpool.tile([128, S], BF, tag="cos")
        sin_s = cpool.tile([128, S], BF, tag="sin")
        tri_s = cpool.tile([128, 128], BF, tag="trim")
        # ones matrix: den matmul broadcasts the k-colsum to all 128
        # partitions, so no cross-partition broadcast hop is needed.
        ones_s = cpool.tile([128, 128], BF, tag="ones")
        nc.gpsimd.memset(ones_s[:], 1.0)
        # NOTE: no PE warmup matmuls — measured counterproductive. The
        # startup phase is DMA-bound; cold (1.2GHz) chains consume the
        # arriving pieces more smoothly, while a pre-warmed PE drains
        # them 2x faster and the resulting idle gaps re-throttle HAM.
        kt = [kvpool.tile([128, S], BF, tag=f"kt{h}", name=f"kt{h}")
              for h in range(HPC)]
        v_s = kvpool.tile([128, NT * 512], BF, tag="v")

        for qc in range(NCH):
            ch = slice(qc * 512, (qc + 1) * 512)
            xt = xpool.tile([128, ND * 512], BF, tag="xt", name=f"xt{qc}")
            if qc == 0:
                # Startup loads. wq/wk are head-major images so head m's
                # chain only needs its own 512KB piece; xt pieces land
                # right behind wq-m0 so the first chain is fed in order.
                # Two HWDGE FIFOs in parallel: x pieces + rope tables on
                # the scalar queue, weights on the sync queue, so the Q
                # chains' inputs arrive in roughly half the time.
                for p in range(4):
                    nc.scalar.dma_start(xt[:, p * 2048:(p + 1) * 2048],
                                        xr[0:128, p * 2048:(p + 1) * 2048])
                nc.scalar.dma_start(cos_s[:], cos2[:])
                nc.scalar.dma_start(sin_s[:], sin2[:])
                nc.scalar.dma_start(tri_s[:], trim[:])
                for m in range(4):
                    nc.sync.dma_start(wq_s[:, m * 2048:(m + 1) * 2048],
                                      wqr[:, m * 2048:(m + 1) * 2048])
                for m in range(4):
                    nc.sync.dma_start(wk_s[:, m * 2048:(m + 1) * 2048],
                                      wkr[:, m * 2048:(m + 1) * 2048])
            else:
                nc.sync.dma_start(xt[:], xr[qc * 128:(qc + 1) * 128, :])

            # ---- per-chunk emission helpers ----------------------------
            noff = 4 * qc
            qt = []
            ctxT = []
            att = {}  # per-head attention state: (ctx_ps, accq)

            def emit_proj(m, is_q):
                w_s = wq_s if is_q else wk_s
                nm = "q" if is_q else "k"
                ps = peps.tile([128, 512], F32, tag="peps",
                               name=f"ps{nm}{qc}_{m}")
                for d in range(ND):
                    nc.tensor.matmul(
                        ps[:],
                        w_s[:, m * 2048 + d * 128:m * 2048 + (d + 1) * 128],
                        xt[:, d * 512:(d + 1) * 512],
                        start=(d == 0), stop=(d == ND - 1))
                raw = rpool.tile([128, 512], BF, tag="rraw",
                                 name=f"raw{nm}{qc}_{m}")
                nc.scalar.copy(raw[:], ps[:])
                # rope rotate-half: swap 64-partition halves via DMA
                swp = rpool.tile([128, 512], BF, tag="rswp",
                                 name=f"swp{nm}{qc}_{m}")
                nc.sync.dma_start(swp[0:64, :], raw[64:128, :])
                nc.sync.dma_start(swp[64:128, :], raw[0:64, :])
                t1 = rpool.tile([128, 512], BF, tag="rt1")
                nc.vector.tensor_mul(t1[:], raw[:], cos_s[:, ch])
                nc.vector.tensor_mul(swp[:], swp[:], sin_s[:, ch])
                if is_q:
                    dst = qpool.tile([128, 512], BF, tag=f"qt{m}",
                                     name=f"qt{m}_{qc}")
                    qt.append(dst)
                    nc.vector.tensor_add(dst[:], t1[:], swp[:])
                else:
                    nc.vector.tensor_add(kt[m][:, ch], t1[:], swp[:])

            def emit_v(tt):
                pv = peps.tile([128, 512], F32, tag="peps",
                               name=f"psv{qc}_{tt}")
                for d in range(ND):
                    nc.tensor.matmul(
                        pv[:],
                        xt[:, d * 512 + tt * 128:d * 512 + (tt + 1) * 128],
                        wv_s[:, d * 512:(d + 1) * 512],
                        start=(d == 0), stop=(d == ND - 1))
                j = qc * 4 + tt
                nc.vector.tensor_copy(v_s[:, j * 512:(j + 1) * 512], pv[:])

            def emit_off(h):
                # off-diagonal k-tiles (full 512-wide, no mask)
                ctx_ps = ctxps_pool.tile([128, 512], F32, tag="ctxps",
                                         name=f"ctxps{h}_{qc}")
                den_ps = denpool.tile([128, 512], F32, tag="den",
                                      name=f"den{h}_{qc}")
                att[h] = (ctx_ps, den_ps)
                pg_hist = {}
                pair = [None, None]
                accq = None
                for j in range(noff):
                    sc = scpool.tile([128, 512], F32, tag="sc",
                                     name=f"sc{h}_{qc}_{j}")
                    nc.tensor.matmul(
                        sc[:], kt[h][:, j * 128:(j + 1) * 128],
                        qt[h][:], start=True, stop=True)
                    pg = ppool.tile([128, 512], BF, tag="pg",
                                    name=f"pg{h}_{qc}_{j}")
                    nc.scalar.activation(pg[:], sc[:], Exp, scale=SCALE)
                    nc.tensor.matmul(
                        ctx_ps[:],
                        v_s[:, j * 512 + h * 128:j * 512 + (h + 1) * 128],
                        pg[:], start=(j == 0), stop=False)
                    # denominator pre-sums on DVE: pairs -> quads -> fold
                    pg_hist[j] = pg
                    if j % 2 == 1:
                        pp = pspool.tile([128, 512], BF,
                                         tag=f"pgs{(j // 2) % 2}",
                                         name=f"pgs{h}_{qc}_{j}")
                        nc.vector.tensor_add(pp[:], pg_hist[j - 1][:], pg[:])
                        pair[(j // 2) % 2] = pp
                    if j % 4 == 3:
                        p4 = pspool.tile([128, 512], BF, tag="pgs4",
                                         name=f"pgs4_{h}_{qc}_{j}")
                        nc.vector.tensor_add(p4[:], pair[0][:], pair[1][:])
                        if accq is None:
                            accq = p4
                        else:
                            nacc = pspool.tile([128, 512], BF, tag="pgs8",
                                               name=f"pgs8_{h}_{qc}_{j}")
                            nc.vector.tensor_add(nacc[:], accq[:], p4[:])
                            accq = nacc
                if accq is not None:
                    nc.tensor.matmul(den_ps[:], ones_s[:], accq[:],
                                     start=True, stop=False)

            def emit_diag(h):
                # diagonal k-tiles, causally trimmed: k-tile r only sees
                # q columns >= 128r; the first 128 of those are the
                # triangular boundary block (element mask).
                ctx_ps, den_ps = att[h]
                p4d = dpool.tile([128, 512], BF, tag="p4d",
                                 name=f"p4d{h}_{qc}")
                for r in range(4):
                    off = 128 * r
                    j = noff + r
                    sc = scpool.tile([128, 512], F32, tag="sc",
                                     name=f"scd{h}_{qc}_{r}")
                    nc.tensor.matmul(
                        sc[:, off:512], kt[h][:, j * 128:(j + 1) * 128],
                        qt[h][:, off:512], start=True, stop=True)
                    pg = ppool.tile([128, 512], BF, tag="pg",
                                    name=f"pgd{h}_{qc}_{r}")
                    nc.scalar.activation(pg[:, off:512], sc[:, off:512],
                                         Exp, scale=SCALE)
                    nc.vector.tensor_mul(pg[:, off:off + 128],
                                         pg[:, off:off + 128], tri_s[:])
                    nc.tensor.matmul(
                        ctx_ps[:, off:512],
                        v_s[:, j * 512 + h * 128:j * 512 + (h + 1) * 128],
                        pg[:, off:512],
                        start=(qc == 0 and r == 0), stop=(r == 3))
                    if r == 0:
                        nc.vector.tensor_copy(p4d[:], pg[:])
                    else:
                        nc.vector.tensor_add(p4d[:, off:512],
                                             p4d[:, off:512], pg[:, off:512])
                nc.tensor.matmul(den_ps[:], ones_s[:], p4d[:],
                                 start=(qc == 0), stop=True)
                # softmax normalization folded into ctx eviction; den is
                # already broadcast across partitions by the ones matmul
                rbc = spool.tile([128, 512], F32, tag="rbc")
                nc.vector.reciprocal_approx_fast(rbc[:], den_ps[:])
                ctx = ctxpool.tile([128, 512], BF, tag=f"ctx{h}",
                                   name=f"ctxT{h}_{qc}")
                ctxT.append(ctx)
                nc.vector.tensor_mul(ctx[:], ctx_ps[:], rbc[:])

            # ---- emission order ----------------------------------------
            if qc < NCH - 1:
                for m in range(HPC):
                    emit_proj(m, True)
                    emit_proj(m, False)
                if qc == 0:
                    # wv/wo are needed later; issue after the Q/K chains so
                    # chunk-0 rope swaps aren't stuck behind them on the queue
                    nc.sync.dma_start(wv_s[:, 0:4096], wvr[:, 0:4096])
                    nc.sync.dma_start(wv_s[:, 4096:8192], wvr[:, 4096:8192])
                    nc.sync.dma_start(wo_s[:], wor[:])
                for tt in range(4):
                    emit_v(tt)
                # prefetch next chunk's x while attention runs
                xt_next = xpool.tile([128, ND * 512], BF, tag="xt",
                                     name=f"xt{qc + 1}")
                nc.sync.dma_start(
                    xt_next[:], xr[(qc + 1) * 128:(qc + 2) * 128, :])
                for h in range(HPC):
                    emit_off(h)
                    emit_diag(h)
            else:
                # last chunk: no next-chunk projections exist to hide the
                # ACT-bound softmax, so interleave this chunk's own K/V
                # chains with the off-diagonal attention of heads 0/1
                for m in range(HPC):
                    emit_proj(m, True)
                emit_proj(0, False)
                emit_proj(1, False)
                emit_off(0)
                emit_proj(2, False)
                emit_proj(3, False)
                emit_off(1)
                for tt in range(4):
                    emit_v(tt)
                emit_diag(0)
                emit_diag(1)
                emit_off(2)
                emit_off(3)
                emit_diag(2)
                emit_diag(3)

            # ---- output projection for chunk qc ------------------------
            # Groups are staggered: each group's first 3 head-MMs are
            # emitted two groups ahead of its final head-3 MM, so the PE
            # has ready work while the last head's softmax tail (den ->
            # recip -> normalize) resolves.
            STAG = 2 if qc < NCH - 1 else 3
            groups = [(tl, dc) for tl in range(4) for dc in range(4)]
            opst = {}
            osbt = {}
            for gi in range(16 + STAG):
                if gi >= STAG:
                    tl, dc = groups[gi - STAG]
                    tt = 4 * qc + tl
                    ops = opst[gi - STAG]
                    osb = osbt[tl]
                    nc.tensor.matmul(
                        ops[:], ctxT[3][:, tl * 128:(tl + 1) * 128],
                        wo_s[:, 3 * D + dc * 512:3 * D + (dc + 1) * 512],
                        start=False, stop=True)
                    if dc % 2 == 0:
                        nc.vector.tensor_copy(
                            osb[:, dc * 512:(dc + 1) * 512], ops[:])
                    else:
                        nc.scalar.copy(
                            osb[:, dc * 512:(dc + 1) * 512], ops[:])
                    # write each half-row as soon as its evacs are done
                    if dc == 1:
                        nc.sync.dma_start(
                            out[tt * 128:(tt + 1) * 128, 0:1024],
                            osb[:, 0:1024])
                    elif dc == 3:
                        nc.sync.dma_start(
                            out[tt * 128:(tt + 1) * 128, 1024:2048],
                            osb[:, 1024:2048])
                if gi < 16:
                    tl, dc = groups[gi]
                    if dc == 0:
                        osbt[tl] = outpool.tile([128, D], BF, tag="osb",
                                                name=f"osb{4 * qc + tl}")
                    # ops tiles live in the scores pool (disjoint in time
                    # with attention) so proj(qc+1) never waits on wo
                    # slots; on the last chunk use the idle proj pool
                    # instead, leaving the sc pool to head-3's scores so
                    # wo partials can overlap the tail of attention.
                    if qc == NCH - 1:
                        ops = peps.tile([128, 512], F32, tag="peps",
                                        name=f"ops{qc}_{tl}_{dc}")
                    else:
                        ops = scpool.tile([128, 512], F32, tag="sc",
                                          name=f"ops{qc}_{tl}_{dc}")
                    opst[gi] = ops
                    for e in range(3):
                        nc.tensor.matmul(
                            ops[:], ctxT[e][:, tl * 128:(tl + 1) * 128],
                            wo_s[:, e * D + dc * 512:e * D + (dc + 1) * 512],
                            start=(e == 0), stop=False)


def _host_prep(x, freqs_cos, freqs_sin, mask, wq, wk, wv, wo):
    """Build per-core input dicts (SBUF-image layouts, bf16)."""
    x = np.asarray(x, np.float32)
    wq = np.asarray(wq, np.float32)
    wk = np.asarray(wk, np.float32)
    wv = np.asarray(wv, np.float32)
    wo = np.asarray(wo, np.float32)
    cos = np.asarray(freqs_cos, np.float32)
    sin = np.asarray(freqs_sin, np.float32)

    perm = np.concatenate([np.arange(0, HD, 2), np.arange(1, HD, 2)])
    cos2 = np.ascontiguousarray(
        np.concatenate([cos.T, cos.T], axis=0)).astype(BF16)
    sin2 = np.ascontiguousarray(
        np.concatenate([-sin.T, sin.T], axis=0)).astype(BF16)

    # triangular boundary mask: T[kl, ql] = 1 iff ql >= kl
    kl = np.arange(128)[:, None]
    ql = np.arange(128)[None, :]
    trim = (ql >= kl).astype(np.float32).astype(BF16)

    def img_dxk(wT):
        # [D, K] -> [128, ND*K] with [p, d*K+c] = wT[d*128+p, c]
        Dd, K = wT.shape
        return np.ascontiguousarray(
            wT.reshape(ND, 128, K).transpose(1, 0, 2).reshape(128, ND * K))

    def img_head_major(wT):
        # [D, 512] -> [128, 8192] with [p, m*2048 + d*128 + c] =
        # wT[d*128+p, m*128+c]; head m's chain reads a contiguous 512KB
        return np.ascontiguousarray(
            wT.reshape(ND, 128, HPC, 128).transpose(1, 2, 0, 3).reshape(
                128, ND * 512))

    in_maps = []
    for c in range(NCORE):
        b = c // 4
        o0 = OSL * (c % 4)
        rows = np.concatenate(
            [o0 + h * HD + perm for h in range(HPC)])
        xT = np.ascontiguousarray(x[b].T)  # [D, S]
        # x image: [4*128, ND*512]: [qc*128+p, d*512+t] = xT[d*128+p, qc*512+t]
        xi = xT.reshape(ND, 128, 4, 512).transpose(2, 1, 0, 3).reshape(
            4 * 128, ND * 512)
        # wo image: [128, HPC*D]: [p, e*D+c] = woT[e*128+p, c]
        woT = wo[:, o0:o0 + OSL].T  # [512, D]
        woi = woT.reshape(HPC, 128, D).transpose(1, 0, 2).reshape(128, HPC * D)
        in_maps.append(dict(
            xr=np.ascontiguousarray(xi).astype(BF16),
            wqr=img_head_major(wq[rows].T).astype(BF16),
            wkr=img_head_major(wk[rows].T).astype(BF16),
            wvr=img_dxk(wv[o0:o0 + OSL].T).astype(BF16),
            wor=np.ascontiguousarray(woi).astype(BF16),
            cos2=cos2, sin2=sin2, trim=trim,
        ))
    return in_maps


_NC_CACHE = None


def get_program():
    global _NC_CACHE
    if _NC_CACHE is None:
        _NC_CACHE = _build_program()
    return _NC_CACHE


def run_on_cores(in_maps, trace=False):
    nc = get_program()
    return bass_utils.run_bass_kernel_spmd(
        nc, in_maps, core_ids=list(range(NCORE)), trace=trace)


def kernel(x, freqs_cos, freqs_sin, mask, wq, wk, wv, wo, start_pos=0,
           **_ignored):
    in_maps = _host_prep(x, freqs_cos, freqs_sin, mask, wq, wk, wv, wo)
    res = run_on_cores(in_maps, trace=False)
    outs = [res.results[c]["out"] for c in range(NCORE)]
    full = np.empty((B, S, D), np.float32)
    for b in range(B):
        acc = outs[4 * b].astype(np.float32)
        for c in range(4 * b + 1, 4 * b + 4):
            acc = acc + outs[c]
        full[b] = acc
    return full


# revision 38
# speedup vs baseline: 1.3800x; 1.0073x over previous
"""Trainium2 Bass kernel for causal multi-head attention with RoPE.

Problem: B=2, S=2048, D=2048, H=16 heads (HD=128), fp32 reference.

Sharding (8 NeuronCores): 2-way batch x 4-way heads. Core c handles
batch c//4 and heads 4*(c%4) .. 4*(c%4)+4. Each core computes a partial
output projection over its 512-wide head slice; the host sums the 4
partials per batch element (the row-parallel wo all-reduce).

v2 changes vs the 370us baseline (all aimed at DMA efficiency + PE
streaming):
  - All inputs repacked host-side into SBUF-image layouts so every DMA
    moves 4-16KB contiguous per partition line (vs 1KB before) in a
    handful of dma_starts (vs 259): each dma_start costs ~610ns of
    serial SyncE issue time and 1KB packets cap HBM at ~190GB/s.
  - Output written as full 128-token rows [128, 2048] (4KB lines).
  - Causal trim of the diagonal super-block: scores/exp/PV only touch
    the valid q-range of each diagonal k-tile (saves ~10us PE, ~20us
    ACT exp); one shared [128,128] triangular mask replaces the
    [128,4,512] binary mask.
  - Per-chunk pooled q/ctx tiles; PSUM banks: proj+wo share 3, scores
    2, ctx 2, denominator 1.
"""

import math

import numpy as np
import ml_dtypes

import concourse.bass as bass
import concourse.mybir as mybir
import concourse.tile as tile
from concourse import bacc, bass_isa, bass_utils

BF16 = ml_dtypes.bfloat16
F32 = mybir.dt.float32
BF = mybir.dt.bfloat16

B, S, D, H = 2, 2048, 2048, 16
HD = 128
NCORE = 8
HPC = 4            # heads per core
OSL = HPC * HD     # 512-wide output slice per core
NT = S // 128      # 16 token tiles
ND = D // 128      # 16 contraction tiles
NCH = 4            # 512-wide token chunks
SCALE = 1.0 / math.sqrt(HD)


def _build_program():
    nc = bacc.Bacc(
        "TRN2",
        target_bir_lowering=False,
        debug=False,
        enable_asserts=False,
        num_devices=NCORE,
    )
    # All images are laid out so a DMA line (per-partition contiguous
    # run) is >=4KB: [128, K] with K*2B contiguous per partition.
    xr = nc.dram_tensor("xr", [4 * 128, ND * 512], BF, kind="ExternalInput").ap()
    wqr = nc.dram_tensor("wqr", [128, ND * 512], BF, kind="ExternalInput").ap()
    wkr = nc.dram_tensor("wkr", [128, ND * 512], BF, kind="ExternalInput").ap()
    wvr = nc.dram_tensor("wvr", [128, ND * 512], BF, kind="ExternalInput").ap()
    wor = nc.dram_tensor("wor", [128, HPC * D], BF, kind="ExternalInput").ap()
    cos2 = nc.dram_tensor("cos2", [128, S], BF, kind="ExternalInput").ap()
    sin2 = nc.dram_tensor("sin2", [128, S], BF, kind="ExternalInput").ap()
    trim = nc.dram_tensor("trim", [128, 128], BF, kind="ExternalInput").ap()
    out = nc.dram_tensor("out", [S, D], BF, kind="ExternalOutput").ap()

    with tile.TileContext(nc) as tc:
        _kernel_body(tc, xr, wqr, wkr, wvr, wor, cos2, sin2, trim, out)
    nc.compile()
    return nc


def _kernel_body(tc, xr, wqr, wkr, wvr, wor, cos2, sin2, trim, out):
    nc = tc.nc
    Exp = mybir.ActivationFunctionType.Exp

    with (
        tc.tile_pool(name="weights", bufs=1) as wpool,
        tc.tile_pool(name="kv", bufs=1) as kvpool,
        tc.tile_pool(name="consts", bufs=1) as cpool,
        tc.tile_pool(name="qchunk", bufs=2) as qpool,
        tc.tile_pool(name="ctxsb", bufs=2) as ctxpool,
        tc.tile_pool(name="xtp", bufs=2) as xpool,
        tc.tile_pool(name="rope", bufs=3) as rpool,
        tc.tile_pool(name="pg", bufs=6) as ppool,
        tc.tile_pool(name="pgs", bufs=2) as pspool,
        tc.tile_pool(name="den", bufs=2) as dpool,
        tc.tile_pool(name="smallsb", bufs=3) as spool,
        tc.tile_pool(name="outsb", bufs=2) as outpool,
        tc.tile_pool(name="peps", bufs=3, space="PSUM") as peps,
        tc.tile_pool(name="scps", bufs=2, space="PSUM") as scpool,
        tc.tile_pool(name="denps", bufs=1, space="PSUM") as denpool,
        tc.tile_pool(name="ctxps", bufs=2, space="PSUM") as ctxps_pool,
    ):
        wq_s = wpool.tile([128, ND * 512], BF, tag="wq")
        wk_s = wpool.tile([128, ND * 512], BF, tag="wk")
        wv_s = wpool.tile([128, ND * 512], BF, tag="wv")
        wo_s = wpool.tile([128, HPC * D], BF, tag="wo")
        cos_s = cpool.tile([128, S], BF, tag="cos")
        sin_s = cpool.tile([128, S], BF, tag="sin")
        tri_s = cpool.tile([128, 128], BF, tag="trim")
        # ones matrix: den matmul broadcasts the k-colsum to all 128
        # partitions, so no cross-partition broadcast hop is needed.
        ones_s = cpool.tile([128, 128], BF, tag="ones")
        nc.gpsimd.memset(ones_s[:], 1.0)
        # NOTE: no PE warmup matmuls — measured counterproductive. The
        # startup phase is DMA-bound; cold (1.2GHz) chains consume the
        # arriving pieces more smoothly, while a pre-warmed PE drains
        # them 2x faster and the resulting idle gaps re-throttle HAM.
        kt = [kvpool.tile([128, S], BF, tag=f"kt{h}", name=f"kt{h}")
              for h in range(HPC)]
        v_s = kvpool.tile([128, NT * 512], BF, tag="v")

        for qc in range(NCH):
            ch = slice(qc * 512, (qc + 1) * 512)
            xt = xpool.tile([128, ND * 512], BF, tag="xt", name=f"xt{qc}")
            if qc == 0:
                # Startup loads. wq/wk are head-major images so head m's
                # chain only needs its own 512KB piece; xt pieces land
                # right behind wq-m0 so the first chain is fed in order.
                # FIFO delivery order paces the chains: head m's weights
                # right before the x pieces its predecessor is waiting on
                nc.sync.dma_start(wq_s[:, 0:2048], wqr[:, 0:2048])
                nc.sync.dma_start(xt[:, 0:2048], xr[0:128, 0:2048])
                nc.sync.dma_start(xt[:, 2048:4096], xr[0:128, 2048:4096])
                nc.sync.dma_start(wq_s[:, 2048:4096], wqr[:, 2048:4096])
                nc.sync.dma_start(xt[:, 4096:6144], xr[0:128, 4096:6144])
                nc.sync.dma_start(wq_s[:, 4096:6144], wqr[:, 4096:6144])
                nc.sync.dma_start(xt[:, 6144:8192], xr[0:128, 6144:8192])
                nc.sync.dma_start(wq_s[:, 6144:8192], wqr[:, 6144:8192])
                for m in range(4):
                    nc.sync.dma_start(wk_s[:, m * 2048:(m + 1) * 2048],
                                      wkr[:, m * 2048:(m + 1) * 2048])
                nc.sync.dma_start(cos_s[:], cos2[:])
                nc.sync.dma_start(sin_s[:], sin2[:])
                nc.sync.dma_start(tri_s[:], trim[:])
            else:
                nc.sync.dma_start(xt[:], xr[qc * 128:(qc + 1) * 128, :])

            # ---- per-chunk emission helpers ----------------------------
            noff = 4 * qc
            qt = []
            ctxT = []
            att = {}  # per-head attention state: (ctx_ps, accq)

            def emit_proj(m, is_q):
                w_s = wq_s if is_q else wk_s
                nm = "q" if is_q else "k"
                ps = peps.tile([128, 512], F32, tag="peps",
                               name=f"ps{nm}{qc}_{m}")
                for d in range(ND):
                    nc.tensor.matmul(
                        ps[:],
                        w_s[:, m * 2048 + d * 128:m * 2048 + (d + 1) * 128],
                        xt[:, d * 512:(d + 1) * 512],
                        start=(d == 0), stop=(d == ND - 1))
                raw = rpool.tile([128, 512], BF, tag="rraw",
                                 name=f"raw{nm}{qc}_{m}")
                nc.scalar.copy(raw[:], ps[:])
                # rope rotate-half: swap 64-partition halves via DMA
                swp = rpool.tile([128, 512], BF, tag="rswp",
                                 name=f"swp{nm}{qc}_{m}")
                nc.sync.dma_start(swp[0:64, :], raw[64:128, :])
                nc.sync.dma_start(swp[64:128, :], raw[0:64, :])
                t1 = rpool.tile([128, 512], BF, tag="rt1")
                nc.vector.tensor_mul(t1[:], raw[:], cos_s[:, ch])
                nc.vector.tensor_mul(swp[:], swp[:], sin_s[:, ch])
                if is_q:
                    dst = qpool.tile([128, 512], BF, tag=f"qt{m}",
                                     name=f"qt{m}_{qc}")
                    qt.append(dst)
                    nc.vector.tensor_add(dst[:], t1[:], swp[:])
                else:
                    nc.vector.tensor_add(kt[m][:, ch], t1[:], swp[:])

            def emit_v(tt):
                pv = peps.tile([128, 512], F32, tag="peps",
                               name=f"psv{qc}_{tt}")
                for d in range(ND):
                    nc.tensor.matmul(
                        pv[:],
                        xt[:, d * 512 + tt * 128:d * 512 + (tt + 1) * 128],
                        wv_s[:, d * 512:(d + 1) * 512],
                        start=(d == 0), stop=(d == ND - 1))
                j = qc * 4 + tt
                nc.vector.tensor_copy(v_s[:, j * 512:(j + 1) * 512], pv[:])

            def emit_off(h):
                # off-diagonal k-tiles (full 512-wide, no mask)
                ctx_ps = ctxps_pool.tile([128, 512], F32, tag="ctxps",
                                         name=f"ctxps{h}_{qc}")
                den_ps = denpool.tile([128, 512], F32, tag="den",
                                      name=f"den{h}_{qc}")
                att[h] = (ctx_ps, den_ps)
                pg_hist = {}
                pair = [None, None]
                accq = None
                for j in range(noff):
                    sc = scpool.tile([128, 512], F32, tag="sc",
                                     name=f"sc{h}_{qc}_{j}")
                    nc.tensor.matmul(
                        sc[:], kt[h][:, j * 128:(j + 1) * 128],
                        qt[h][:], start=True, stop=True)
                    pg = ppool.tile([128, 512], BF, tag="pg",
                                    name=f"pg{h}_{qc}_{j}")
                    nc.scalar.activation(pg[:], sc[:], Exp, scale=SCALE)
                    nc.tensor.matmul(
                        ctx_ps[:],
                        v_s[:, j * 512 + h * 128:j * 512 + (h + 1) * 128],
                        pg[:], start=(j == 0), stop=False)
                    # denominator pre-sums on DVE: pairs -> quads -> fold
                    pg_hist[j] = pg
                    if j % 2 == 1:
                        pp = pspool.tile([128, 512], BF,
                                         tag=f"pgs{(j // 2) % 2}",
                                         name=f"pgs{h}_{qc}_{j}")
                        nc.vector.tensor_add(pp[:], pg_hist[j - 1][:], pg[:])
                        pair[(j // 2) % 2] = pp
                    if j % 4 == 3:
                        p4 = pspool.tile([128, 512], BF, tag="pgs4",
                                         name=f"pgs4_{h}_{qc}_{j}")
                        nc.vector.tensor_add(p4[:], pair[0][:], pair[1][:])
                        if accq is None:
                            accq = p4
                        else:
                            nacc = pspool.tile([128, 512], BF, tag="pgs8",
                                               name=f"pgs8_{h}_{qc}_{j}")
                            nc.vector.tensor_add(nacc[:], accq[:], p4[:])
                            accq = nacc
                if accq is not None:
                    nc.tensor.matmul(den_ps[:], ones_s[:], accq[:],
                                     start=True, stop=False)

            def emit_diag(h):
                # diagonal k-tiles, causally trimmed: k-tile r only sees
                # q columns >= 128r; the first 128 of those are the
                # triangular boundary block (element mask).
                ctx_ps, den_ps = att[h]
                p4d = dpool.tile([128, 512], BF, tag="p4d",
                                 name=f"p4d{h}_{qc}")
                for r in range(4):
                    off = 128 * r
                    j = noff + r
                    sc = scpool.tile([128, 512], F32, tag="sc",
                                     name=f"scd{h}_{qc}_{r}")
                    nc.tensor.matmul(
                        sc[:, off:512], kt[h][:, j * 128:(j + 1) * 128],
                        qt[h][:, off:512], start=True, stop=True)
                    pg = ppool.tile([128, 512], BF, tag="pg",
                                    name=f"pgd{h}_{qc}_{r}")
                    nc.scalar.activation(pg[:, off:512], sc[:, off:512],
                                         Exp, scale=SCALE)
                    nc.vector.tensor_mul(pg[:, off:off + 128],
                                         pg[:, off:off + 128], tri_s[:])
                    nc.tensor.matmul(
                        ctx_ps[:, off:512],
                        v_s[:, j * 512 + h * 128:j * 512 + (h + 1) * 128],
                        pg[:, off:512],
                        start=(qc == 0 and r == 0), stop=(r == 3))
                    if r == 0:
                        nc.vector.tensor_copy(p4d[:], pg[:])
                    else:
                        nc.vector.tensor_add(p4d[:, off:512],
                                             p4d[:, off:512], pg[:, off:512])
                nc.tensor.matmul(den_ps[:], ones_s[:], p4d[:],
                                 start=(qc == 0), stop=True)
                # softmax normalization folded into ctx eviction; den is
                # already broadcast across partitions by the ones matmul
                rbc = spool.tile([128, 512], F32, tag="rbc")
                nc.vector.reciprocal_approx_fast(rbc[:], den_ps[:])
                ctx = ctxpool.tile([128, 512], BF, tag=f"ctx{h}",
                                   name=f"ctxT{h}_{qc}")
                ctxT.append(ctx)
                nc.vector.tensor_mul(ctx[:], ctx_ps[:], rbc[:])

            # ---- emission order ----------------------------------------
            if qc < NCH - 1:
                for m in range(HPC):
                    emit_proj(m, True)
                    emit_proj(m, False)
                if qc == 0:
                    # wv/wo are needed later; issue after the Q/K chains so
                    # chunk-0 rope swaps aren't stuck behind them on the queue
                    nc.sync.dma_start(wv_s[:, 0:4096], wvr[:, 0:4096])
                    nc.sync.dma_start(wv_s[:, 4096:8192], wvr[:, 4096:8192])
                    nc.sync.dma_start(wo_s[:], wor[:])
                for tt in range(4):
                    emit_v(tt)
                # prefetch next chunk's x while attention runs
                xt_next = xpool.tile([128, ND * 512], BF, tag="xt",
                                     name=f"xt{qc + 1}")
                nc.sync.dma_start(
                    xt_next[:], xr[(qc + 1) * 128:(qc + 2) * 128, :])
                for h in range(HPC):
                    emit_off(h)
                    emit_diag(h)
            else:
                # last chunk: no next-chunk projections exist to hide the
                # ACT-bound softmax, so interleave this chunk's own K/V
                # chains with the off-diagonal attention of heads 0/1
                for m in range(HPC):
                    emit_proj(m, True)
                emit_proj(0, False)
                emit_proj(1, False)
                emit_off(0)
                emit_proj(2, False)
                emit_proj(3, False)
                emit_off(1)
                for tt in range(4):
                    emit_v(tt)
                emit_diag(0)
                emit_diag(1)
                emit_off(2)
                emit_diag(2)
                emit_off(3)
                emit_diag(3)

            # ---- output projection for chunk qc ------------------------
            # Groups are staggered: each group's first 3 head-MMs are
            # emitted two groups ahead of its final head-3 MM, so the PE
            # has ready work while the last head's softmax tail (den ->
            # recip -> normalize) resolves.
            STAG = 2 if qc < NCH - 1 else 4
            groups = [(tl, dc) for tl in range(4) for dc in range(4)]
            opst = {}
            osbt = {}
            for gi in range(16 + STAG):
                if gi >= STAG:
                    tl, dc = groups[gi - STAG]
                    tt = 4 * qc + tl
                    ops = opst[gi - STAG]
                    osb = osbt[tl]
                    nc.tensor.matmul(
                        ops[:], ctxT[3][:, tl * 128:(tl + 1) * 128],
                        wo_s[:, 3 * D + dc * 512:3 * D + (dc + 1) * 512],
                        start=False, stop=True)
                    if dc % 2 == 0:
                        nc.vector.tensor_copy(
                            osb[:, dc * 512:(dc + 1) * 512], ops[:])
                    else:
                        nc.scalar.copy(
                            osb[:, dc * 512:(dc + 1) * 512], ops[:])
                    # write each half-row as soon as its evacs are done
                    if dc == 1:
                        nc.sync.dma_start(
                            out[tt * 128:(tt + 1) * 128, 0:1024],
                            osb[:, 0:1024])
                    elif dc == 3:
                        nc.sync.dma_start(
                            out[tt * 128:(tt + 1) * 128, 1024:2048],
                            osb[:, 1024:2048])
                if gi < 16:
                    tl, dc = groups[gi]
                    if dc == 0:
                        osbt[tl] = outpool.tile([128, D], BF, tag="osb",
                                                name=f"osb{4 * qc + tl}")
                    # ops tiles live in the scores pool (disjoint in time
                    # with attention) so proj(qc+1) never waits on wo
                    # slots; on the last chunk the idle proj pool doubles
                    # the wo pipeline depth.
                    if qc == NCH - 1 and gi % 2 == 1:
                        ops = peps.tile([128, 512], F32, tag="peps",
                                        name=f"ops{qc}_{tl}_{dc}")
                    else:
                        ops = scpool.tile([128, 512], F32, tag="sc",
                                          name=f"ops{qc}_{tl}_{dc}")
                    opst[gi] = ops
                    for e in range(3):
                        nc.tensor.matmul(
                            ops[:], ctxT[e][:, tl * 128:(tl + 1) * 128],
                            wo_s[:, e * D + dc * 512:e * D + (dc + 1) * 512],
                            start=(e == 0), stop=False)


def _host_prep(x, freqs_cos, freqs_sin, mask, wq, wk, wv, wo):
    """Build per-core input dicts (SBUF-image layouts, bf16)."""
    x = np.asarray(x, np.float32)
    wq = np.asarray(wq, np.float32)
    wk = np.asarray(wk, np.float32)
    wv = np.asarray(wv, np.float32)
    wo = np.asarray(wo, np.float32)
    cos = np.asarray(freqs_cos, np.float32)
    sin = np.asarray(freqs_sin, np.float32)

    perm = np.concatenate([np.arange(0, HD, 2), np.arange(1, HD, 2)])
    cos2 = np.ascontiguousarray(
        np.concatenate([cos.T, cos.T], axis=0)).astype(BF16)
    sin2 = np.ascontiguousarray(
        np.concatenate([-sin.T, sin.T], axis=0)).astype(BF16)

    # triangular boundary mask: T[kl, ql] = 1 iff ql >= kl
    kl = np.arange(128)[:, None]
    ql = np.arange(128)[None, :]
    trim = (ql >= kl).astype(np.float32).astype(BF16)

    def img_dxk(wT):
        # [D, K] -> [128, ND*K] with [p, d*K+c] = wT[d*128+p, c]
        Dd, K = wT.shape
        return np.ascontiguousarray(
            wT.reshape(ND, 128, K).transpose(1, 0, 2).reshape(128, ND * K))

    def img_head_major(wT):
        # [D, 512] -> [128, 8192] with [p, m*2048 + d*128 + c] =
        # wT[d*128+p, m*128+c]; head m's chain reads a contiguous 512KB
        return np.ascontiguousarray(
            wT.reshape(ND, 128, HPC, 128).transpose(1, 2, 0, 3).reshape(
                128, ND * 512))

    in_maps = []
    for c in range(NCORE):
        b = c // 4
        o0 = OSL * (c % 4)
        rows = np.concatenate(
            [o0 + h * HD + perm for h in range(HPC)])
        xT = np.ascontiguousarray(x[b].T)  # [D, S]
        # x image: [4*128, ND*512]: [qc*128+p, d*512+t] = xT[d*128+p, qc*512+t]
        xi = xT.reshape(ND, 128, 4, 512).transpose(2, 1, 0, 3).reshape(
            4 * 128, ND * 512)
        # wo image: [128, HPC*D]: [p, e*D+c] = woT[e*128+p, c]
        woT = wo[:, o0:o0 + OSL].T  # [512, D]
        woi = woT.reshape(HPC, 128, D).transpose(1, 0, 2).reshape(128, HPC * D)
        in_maps.append(dict(
            xr=np.ascontiguousarray(xi).astype(BF16),
            wqr=img_head_major(wq[rows].T).astype(BF16),
            wkr=img_head_major(wk[rows].T).astype(BF16),
            wvr=img_dxk(wv[o0:o0 + OSL].T).astype(BF16),
            wor=np.ascontiguousarray(woi).astype(BF16),
            cos2=cos2, sin2=sin2, trim=trim,
        ))
    return in_maps


_NC_CACHE = None


def get_program():
    global _NC_CACHE
    if _NC_CACHE is None:
        _NC_CACHE = _build_program()
    return _NC_CACHE


def run_on_cores(in_maps, trace=False):
    nc = get_program()
    return bass_utils.run_bass_kernel_spmd(
        nc, in_maps, core_ids=list(range(NCORE)), trace=trace)


def kernel(x, freqs_cos, freqs_sin, mask, wq, wk, wv, wo, start_pos=0,
           **_ignored):
    in_maps = _host_prep(x, freqs_cos, freqs_sin, mask, wq, wk, wv, wo)
    res = run_on_cores(in_maps, trace=False)
    outs = [res.results[c]["out"] for c in range(NCORE)]
    full = np.empty((B, S, D), np.float32)
    for b in range(B):
        acc = outs[4 * b].astype(np.float32)
        for c in range(4 * b + 1, 4 * b + 4):
            acc = acc + outs[c]
        full[b] = acc
    return full
